# revision 20
# baseline (speedup 1.0000x reference)
"""DeepseekV3 MLA attention kernel for 8 Trainium2 NeuronCores — v2.

Sharding: 2-way data-parallel over batch x 4-way tensor-parallel over heads.
Core c handles batch b = c // 4 and heads [4*(c%4) .. 4*(c%4)+4).

Design vs the original fp32r baseline:
  - all projection / AV / w_o matmuls in bf16 (half the SBUF/DMA traffic,
    FWL weight loads, 2x DVE elementwise)
  - score matmuls in fp8e4 with DoubleRow perf mode: nope(128) + rope(64,
    zero-padded) packed as a 2-plane 256-deep contraction -> one matmul
    at 0.5 cycles/row
  - kv latent sharded: each core computes a 128-col slice of kv_c, then an
    AllGather over each 4-core batch group assembles the full 512 (kills
    the 4x-redundant kv_c compute, ~41us of PE per core)
  - softmax denominators via DVE running-sums of e-tiles + ones-matmuls
    per (head, q-tile) instead of a ones-matmul per (head, k-tile)
  - causal diag shrink: score/exp/AV restricted to the unmasked column
    range on diagonal tiles
  - q/k/v SBUF-resident; only the kv AllGather round-trips DRAM
  - phases software-pipelined per 512-wide s-tile
    (p1(st) -> p2(st) -> attention(qt=st)), pools hoisted out of the rep
    loop so consecutive reps pipeline into each other

All tolerances validated numerically on the CPU reference: bf16-everything
gives 4.3e-3 scale-relative max error; fp8 q/k adds ~6.7e-3 (tolerance 2e-2).
"""

from contextlib import ExitStack
from dataclasses import dataclass

import numpy as np
import ml_dtypes

import concourse.bacc as bacc
import concourse.mybir as mybir
import concourse.tile as tile

F32 = mybir.dt.float32
BF16 = mybir.dt.bfloat16
FP8 = mybir.dt.float8e4


@dataclass(frozen=True)
class Cfg:
    S: int = 2048          # sequence length (per batch)
    HID: int = 2048        # hidden dim
    QLR: int = 1536        # q lora rank (host-side only)
    KVLR: int = 512        # kv lora rank
    NH_G: int = 4          # heads per core
    DN: int = 128          # nope dim
    DR: int = 64           # rope dim
    DV: int = 128          # v head dim
    ST: int = 512          # phase-1/2 s-tile width
    QT: int = 512          # attention q-tile width

    @property
    def QFN(self):
        return self.NH_G * self.DN      # fused q nope cols (512)

    @property
    def QFR(self):
        return (self.NH_G // 2) * 128   # fused q rope cols, pair-packed (256)

    @property
    def SCALE(self):
        return 1.0 / float(np.sqrt(self.DN + self.DR))


CFG = Cfg()


def build_nc(C: Cfg, reps: int = 1):
    nc = bacc.Bacc("TRN2", target_bir_lowering=False, debug=False, num_devices=8)
    P = 128
    HO = C.HID // P          # 16
    NS = C.S // C.ST         # 4
    KVC = C.KVLR // P        # 4
    NPAIR = C.NH_G // 2      # 2
    NQT = C.S // C.QT        # 4
    NDIAG = C.QT // P        # 4
    NVS = C.S // P           # 16
    NOT = C.HID // 512       # 4
    DV = C.DV

    # ---- kernel I/O ----
    hT = nc.dram_tensor("hT", [C.HID, C.S], BF16, kind="ExternalInput").ap()
    w_qf = nc.dram_tensor("w_qf", [C.HID, C.QFN + C.QFR], BF16,
                          kind="ExternalInput").ap()
    # per-core 128-col slice of w_kv_a; the kv latent is AllGathered
    # across the 4 cores of each batch group
    w_kva = nc.dram_tensor("w_kva", [C.HID, 128], BF16,
                           kind="ExternalInput").ap()
    w_kbn = nc.dram_tensor("w_kbn", [C.KVLR, C.QFN], BF16,
                           kind="ExternalInput").ap()
    w_kbr = nc.dram_tensor("w_kbr", [C.KVLR, C.QFR], BF16,
                           kind="ExternalInput").ap()
    w_vb = nc.dram_tensor("w_vb", [C.KVLR, C.NH_G * DV], BF16,
                          kind="ExternalInput").ap()
    w_ob = nc.dram_tensor("w_ob", [C.NH_G * DV, C.HID], BF16,
                          kind="ExternalInput").ap()
    cos2 = nc.dram_tensor("cos2", [P, C.S], BF16, kind="ExternalInput").ap()
    ssin2 = nc.dram_tensor("ssin2", [P, C.S], BF16, kind="ExternalInput").ap()
    dmask = nc.dram_tensor("dmask", [C.QT, C.QT], BF16, kind="ExternalInput").ap()
    outp = nc.dram_tensor("outp", [C.S, C.HID], BF16, kind="ExternalOutput").ap()

    hT_r = hT.rearrange("(ho hi) s -> hi ho s", hi=P)
    GROUPS = [[0, 1, 2, 3], [4, 5, 6, 7]]

    with tile.TileContext(nc) as tc:
        with ExitStack() as tctx:
            per = tctx.enter_context(tc.tile_pool(name="per", bufs=1))
            ht_pool = tctx.enter_context(tc.tile_pool(name="ht", bufs=2))
            kv_pool = tctx.enter_context(tc.tile_pool(name="kv", bufs=2))
            rp_pool = tctx.enter_context(tc.tile_pool(name="rp", bufs=2))
            e_pool = tctx.enter_context(tc.tile_pool(name="e", bufs=6))
            es_pool = tctx.enter_context(tc.tile_pool(name="es", bufs=2))
            d_pool = tctx.enter_context(tc.tile_pool(name="d", bufs=2))
            ao_pool = tctx.enter_context(tc.tile_pool(name="ao", bufs=2))
            oev_pool = tctx.enter_context(tc.tile_pool(name="oe", bufs=2))
            psA = tctx.enter_context(
                tc.tile_pool(name="psA", bufs=2, space="PSUM"))
            ps_s = tctx.enter_context(
                tc.tile_pool(name="pss", bufs=2, space="PSUM"))
            ps_d = tctx.enter_context(
                tc.tile_pool(name="psd", bufs=1, space="PSUM"))
            ps_o = tctx.enter_context(
                tc.tile_pool(name="pso", bufs=2, space="PSUM"))
            ps_w = tctx.enter_context(
                tc.tile_pool(name="psw", bufs=1, space="PSUM"))
            for rep in range(reps):
                # persistent tiles
                cos_sb = per.tile([P, C.S], BF16)
                ssin_sb = per.tile([P, C.S], BF16)
                dm_sb = per.tile([P, NDIAG, C.QT], BF16)
                ones_sb = per.tile([P, P], BF16)
                wqf_sb = [per.tile([P, C.QFN + C.QFR], BF16, tag=f"wqf{ho}",
                                   name=f"wqf{ho}") for ho in range(HO)]
                wkva_sb = [per.tile([P, P], BF16, tag=f"wkva{ho}",
                                    name=f"wkva{ho}") for ho in range(HO)]
                kvp_d = nc.dram_tensor(f"kvp_scr{rep}", [P, C.S], BF16).ap()
                kvag_d = nc.dram_tensor(f"kvag_scr{rep}", [KVC, P, C.S],
                                        BF16).ap()
                wkn_sb = per.tile([P, KVC, C.QFN], BF16)
                wkr_sb = per.tile([P, KVC, C.QFR], BF16)
                wv_sb = per.tile([P, KVC, C.NH_G * DV], BF16)
                wo_sb = per.tile([P, C.NH_G, C.HID], BF16)
                # q/k in fp8, 2 planes: 0 = nope, 1 = rope (pair-packed)
                qT_sb = per.tile([P, C.NH_G, 2, C.S], FP8)
                kT_sb = per.tile([P, C.NH_G, 2, C.S], FP8)
                v_sb = per.tile([P, NVS, C.NH_G * DV], BF16)

                # hT prefetch: st=0 queued before any weight DMA so the
                # first accumulation can start immediately
                ht_tiles = {}

                def load_ht(st):
                    t = ht_pool.tile([P, HO, C.ST], BF16, tag="ht")
                    s = slice(st * C.ST, (st + 1) * C.ST)
                    for ho in range(0, HO, 2):
                        nc.sync.dma_start(
                            out=t[:, ho:ho + 2, :], in_=hT_r[:, ho:ho + 2, s])
                    ht_tiles[st] = t

                load_ht(0)
                wqf_r = w_qf.rearrange("(ho hi) c -> hi ho c", hi=P)
                wkva_r = w_kva.rearrange("(ho hi) c -> hi ho c", hi=P)
                for ho in range(HO):
                    nc.sync.dma_start(out=wqf_sb[ho][:], in_=wqf_r[:, ho, :])
                for ho in range(HO):
                    nc.sync.dma_start(out=wkva_sb[ho][:], in_=wkva_r[:, ho, :])
                nc.sync.dma_start(out=cos_sb[:], in_=cos2)
                nc.sync.dma_start(out=ssin_sb[:], in_=ssin2)
                nc.gpsimd.memset(ones_sb[:], 1.0)
                # zero the unused rope half of each head's q plane 1 (the k
                # plane 1 keeps the full head pair; the q-side zeros select
                # this head's rope rows in the DoubleRow contraction)
                for h in range(C.NH_G):
                    if h % 2 == 0:
                        nc.gpsimd.memset(qT_sb[64:128, h, 1, :], 0.0)
                    else:
                        nc.gpsimd.memset(qT_sb[0:64, h, 1, :], 0.0)
                def rope_block(ps_nat, s0, dsts):
                    """RoPE a pair-packed psum block [128, ST] (2 heads x 64
                    rope dims); write f32 results to each (dst_ap, r0, r1)."""
                    tmp = rp_pool.tile([P, C.ST], F32, tag="rtmp")
                    nc.vector.tensor_copy(tmp[:], ps_nat[:])
                    qs = rp_pool.tile([P, C.ST], F32, tag="rqs")
                    for g in range(4):
                        nc.sync.dma_start(
                            out=qs[(g ^ 1) * 32:(g ^ 1) * 32 + 32, :],
                            in_=tmp[g * 32:(g + 1) * 32, :])
                    m1 = rp_pool.tile([P, C.ST], F32, tag="rm1")
                    nc.vector.tensor_mul(m1[:], tmp[:], cos_sb[:, s0:s0 + C.ST])
                    nc.vector.tensor_mul(qs[:], qs[:], ssin_sb[:, s0:s0 + C.ST])
                    for dst_ap, r0, r1 in dsts:
                        nc.vector.tensor_add(dst_ap, m1[r0:r1, :], qs[r0:r1, :])

                # ===== Pass A: this core's 128-col slice of the kv
                # latent, then AllGather the full 512 across the group =====
                for st in range(NS):
                    s0 = st * C.ST
                    sl = slice(s0, s0 + C.ST)
                    ht_sb = ht_tiles.pop(st)
                    if st + 1 < NS:
                        load_ht(st + 1)
                    ps = psA.tile([P, C.ST], F32, tag="psA", name="ps")
                    for h in range(HO):
                        nc.tensor.matmul(
                            ps[:], wkva_sb[h][:], ht_sb[:, h, :],
                            start=(h == 0), stop=(h == HO - 1))
                    kvp_sb = kv_pool.tile([P, C.ST], BF16, tag="kvp",
                                          name="kvp_sb")
                    nc.vector.tensor_copy(kvp_sb[:], ps[:])
                    nc.sync.dma_start(out=kvp_d[:, sl], in_=kvp_sb[:])
                nc.gpsimd.collective_compute(
                    "AllGather", mybir.AluOpType.bypass,
                    replica_groups=GROUPS,
                    ins=[kvp_d[:]], outs=[kvag_d[:]])
                load_ht(0)

                for st in range(NS):
                    s0 = st * C.ST
                    sl = slice(s0, s0 + C.ST)

                    # ===== Phase 1: q (fused LoRA) from hidden =====
                    ht_sb = ht_tiles.pop(st)
                    if st + 1 < NS:
                        load_ht(st + 1)
                    kv_t = kv_pool.tile([P, KVC, C.ST], BF16)
                    for cc in range(KVC):
                        nc.sync.dma_start(out=kv_t[:, cc, :],
                                          in_=kvag_d[cc, :, sl])

                    def accum(lhs_sb, col0):
                        ps = psA.tile([P, C.ST], F32, tag="psA")
                        for h in range(HO):
                            nc.tensor.matmul(
                                ps[:], lhs_sb[h][:, col0:col0 + P],
                                ht_sb[:, h, :],
                                start=(h == 0), stop=(h == HO - 1))
                        return ps

                    for t in range(C.NH_G):
                        ps = accum(wqf_sb, t * P)
                        nc.vector.tensor_copy(qT_sb[:, t, 0, sl], ps[:])
                    for pr in range(NPAIR):
                        ps = accum(wqf_sb, C.QFN + pr * P)
                        h0, h1 = 2 * pr, 2 * pr + 1
                        rope_block(ps, s0, [
                            (qT_sb[0:64, h0, 1, sl], 0, 64),
                            (qT_sb[64:128, h1, 1, sl], 64, 128),
                        ])

                    if st == 0:
                        nc.sync.dma_start(
                            out=wkn_sb[:],
                            in_=w_kbn.rearrange("(co ci) m -> ci co m", ci=P))
                        nc.sync.dma_start(
                            out=wkr_sb[:],
                            in_=w_kbr.rearrange("(co ci) m -> ci co m", ci=P))
                        nc.sync.dma_start(
                            out=wv_sb[:],
                            in_=w_vb.rearrange("(co ci) m -> ci co m", ci=P))

                    # ===== Phase 2: k / v head projections from kv latent =====
                    for h in range(C.NH_G):
                        ps = psA.tile([P, C.ST], F32, tag="psA")
                        for cc in range(KVC):
                            nc.tensor.matmul(
                                ps[:], wkn_sb[:, cc, h * C.DN:(h + 1) * C.DN],
                                kv_t[:, cc, :],
                                start=(cc == 0), stop=(cc == KVC - 1))
                        nc.vector.tensor_copy(kT_sb[:, h, 0, sl], ps[:])
                    for pr in range(NPAIR):
                        ps = psA.tile([P, C.ST], F32, tag="psA")
                        for cc in range(KVC):
                            nc.tensor.matmul(
                                ps[:], wkr_sb[:, cc, pr * P:(pr + 1) * P],
                                kv_t[:, cc, :],
                                start=(cc == 0), stop=(cc == KVC - 1))
                        h0, h1 = 2 * pr, 2 * pr + 1
                        rope_block(ps, s0, [(kT_sb[:, h0, 1, sl], 0, 128)])
                        nc.vector.tensor_copy(kT_sb[:, h1, 1, sl],
                                           kT_sb[:, h0, 1, sl])
                    for ssub in range(C.ST // P):
                        vs = (s0 + ssub * P) // P
                        ps = psA.tile([P, C.NH_G * DV], F32, tag="psA")
                        for cc in range(KVC):
                            nc.tensor.matmul(
                                ps[:], kv_t[:, cc, ssub * P:(ssub + 1) * P],
                                wv_sb[:, cc, :],
                                start=(cc == 0), stop=(cc == KVC - 1))
                        nc.vector.tensor_copy(v_sb[:, vs, :], ps[:])

                    if st == 0:
                        nc.sync.dma_start(
                            out=wo_sb[:],
                            in_=w_ob.rearrange("(h d) o -> d h o", d=P))
                        nc.sync.dma_start(
                            out=dm_sb[:],
                            in_=dmask.rearrange("(j ki) q -> ki j q", ki=P))

                    # ================= Phase 3: attention (qt = st) ============
                    qt = st
                    q0 = qt * C.QT
                    nkt = (qt + 1) * C.QT // P
                    ao_sb = ao_pool.tile([P, C.NH_G, C.QT], BF16, tag="ao")
                    for h in range(C.NH_G):
                        # single running e-sum: the DVE add chain (~330ns/add)
                        # keeps pace with the exp producer (~740ns/tile)
                        esum = es_pool.tile([P, C.QT], BF16, tag="esA",
                                            name="esum")
                        pso = ps_o.tile([P, C.QT], F32, tag="pso")
                        for kt in range(nkt):
                            j = kt - qt * NDIAG      # >= 0 -> diagonal tile
                            c0 = max(j, 0) * P       # first unmasked column
                            k0 = kt * P
                            pss = ps_s.tile([P, C.QT], F32, tag="pss")
                            nc.tensor.matmul(
                                pss[:, c0:], kT_sb[:, h, :, k0:k0 + P],
                                qT_sb[:, h, :, q0 + c0:q0 + C.QT],
                                start=True, stop=True,
                                perf_mode=mybir.MatmulPerfMode.DoubleRow)
                            e = e_pool.tile([P, C.QT], BF16, tag="e")
                            nc.scalar.activation(
                                e[:, c0:], pss[:, c0:],
                                mybir.ActivationFunctionType.Exp, scale=C.SCALE)
                            if j >= 0:
                                nc.vector.tensor_mul(
                                    e[:, c0:c0 + P], e[:, c0:c0 + P],
                                    dm_sb[:, j, c0:c0 + P])
                            if kt == 0:
                                nc.vector.tensor_copy(esum[:], e[:])
                            else:
                                nc.vector.tensor_add(
                                    esum[:, c0:], esum[:, c0:], e[:, c0:])
                            nc.tensor.matmul(
                                pso[:, c0:], v_sb[:, kt, h * DV:(h + 1) * DV],
                                e[:, c0:],
                                start=(kt == 0), stop=(kt == nkt - 1))
                        psd = ps_d.tile([P, C.QT], F32, tag="psd")
                        nc.tensor.matmul(psd[:], ones_sb[:], esum[:],
                                         start=True, stop=True)
                        rec = d_pool.tile([P, C.QT], F32, tag="rec")
                        nc.vector.reciprocal(rec[:], psd[:])
                        nc.vector.tensor_mul(ao_sb[:, h, :], pso[:], rec[:])
                    for qs in range(C.QT // P):
                        for ot in range(NOT):
                            psw = ps_w.tile([P, 512], F32, tag="psw")
                            for h in range(C.NH_G):
                                nc.tensor.matmul(
                                    psw[:], ao_sb[:, h, qs * P:(qs + 1) * P],
                                    wo_sb[:, h, ot * 512:(ot + 1) * 512],
                                    start=(h == 0), stop=(h == C.NH_G - 1))
                            oev = oev_pool.tile([P, 512], BF16)
                            nc.vector.tensor_copy(oev[:], psw[:])
                            nc.sync.dma_start(
                                out=outp[q0 + qs * P:q0 + (qs + 1) * P,
                                         ot * 512:(ot + 1) * 512],
                                in_=oev[:])

    nc.compile()
    return nc


def rope_tables(C: Cfg):
    """cos2/ssin2 [128, S] bf16: two stacked 64-row blocks (head pairs
    share); ssin has the rotate-half sign baked into the first 32 rows."""
    inv = 1.0 / (10000.0 ** (np.arange(0, C.DR, 2, dtype=np.float64) / C.DR))
    freqs = np.arange(C.S, dtype=np.float64)[:, None] * inv[None, :]  # [S, 32]
    emb = np.concatenate([freqs, freqs], axis=1)  # [S, 64]
    cos = np.cos(emb).T.astype(np.float32)   # [64, S]
    sin = np.sin(emb).T.astype(np.float32)
    ssin = sin.copy()
    ssin[: C.DR // 2] = -ssin[: C.DR // 2]
    cos2 = np.concatenate([cos, cos], axis=0)     # [128, S]
    ssin2 = np.concatenate([ssin, ssin], axis=0)
    bf = lambda x: np.ascontiguousarray(x).astype(ml_dtypes.bfloat16)
    return bf(cos2), bf(ssin2)


_FUSED_CACHE = {}


def _fused_wq(inputs):
    """w_q_a @ [w_q_nope | w_q_rope] for all heads, computed once."""
    key = id(inputs.get("w_q_a"))
    if _FUSED_CACHE.get("key") != key:
        w_q_a = np.asarray(inputs["w_q_a"], dtype=np.float32)
        wn = w_q_a @ np.asarray(inputs["w_q_nope"], dtype=np.float32)
        wr = w_q_a @ np.asarray(inputs["w_q_rope"], dtype=np.float32)
        _FUSED_CACHE.update(key=key, wn=wn, wr=wr)
    return _FUSED_CACHE["wn"], _FUSED_CACHE["wr"]


def host_inputs(C: Cfg, inputs: dict, core: int):
    """Build the per-core input map from full inputs."""
    NH = inputs["w_q_nope"].shape[1] // C.DN
    groups = NH // C.NH_G
    b = core // groups
    g = core % groups
    hs = slice(g * C.NH_G, (g + 1) * C.NH_G)

    bf = lambda x: np.ascontiguousarray(
        np.asarray(x, dtype=np.float32)).astype(ml_dtypes.bfloat16)

    wn_full, wr_full = _fused_wq(inputs)
    w_qfn = wn_full.reshape(C.HID, NH, C.DN)[:, hs].reshape(C.HID, -1)
    w_qfr = wr_full.reshape(C.HID, NH, C.DR)[:, hs].reshape(C.HID, -1)
    w_qf = bf(np.concatenate([w_qfn, w_qfr], axis=1))

    hT = bf(inputs["hidden_states"][b].T)
    w_kva = bf(np.asarray(inputs["w_kv_a"])[:, g * 128:(g + 1) * 128])
    w_kbn = bf(inputs["w_k_nope"].reshape(C.KVLR, NH, C.DN)[:, hs]
               .reshape(C.KVLR, -1))
    w_kbr = bf(inputs["w_k_rope"].reshape(C.KVLR, NH, C.DR)[:, hs]
               .reshape(C.KVLR, -1))
    w_vb = bf(inputs["w_v"].reshape(C.KVLR, NH, C.DV)[:, hs]
              .reshape(C.KVLR, -1))
    w_ob = bf(inputs["w_o"].reshape(NH, C.DV, C.HID)[hs].reshape(-1, C.HID))
    cos2, ssin2 = rope_tables(C)
    cm = np.asarray(inputs["causal_mask"])[0, 0]
    dmask = np.ascontiguousarray(
        cm[-C.QT:, -C.QT:].T.astype(np.float32)).astype(ml_dtypes.bfloat16)
    return {
        "hT": hT, "w_qf": w_qf, "w_kva": w_kva,
        "w_kbn": w_kbn, "w_kbr": w_kbr, "w_vb": w_vb, "w_ob": w_ob,
        "cos2": cos2, "ssin2": ssin2, "dmask": dmask,
    }


_NC_CACHE = {}


def kernel(**inputs) -> np.ndarray:
    from concourse.bass_utils import run_bass_kernel_spmd

    C = CFG
    if "nc" not in _NC_CACHE:
        _NC_CACHE["nc"] = build_nc(C)
    nc = _NC_CACHE["nc"]

    in_maps = [host_inputs(C, inputs, c) for c in range(8)]
    res = run_bass_kernel_spmd(nc, in_maps, core_ids=list(range(8)))

    B = inputs["hidden_states"].shape[0]
    groups = 8 // B
    out = np.zeros((B, C.S, C.HID), dtype=np.float32)
    for c in range(8):
        out[c // groups] += np.asarray(res.results[c]["outp"],
                                       dtype=np.float32)
    return out
